# revision 1
# baseline (speedup 1.0000x reference)
"""GCN (2-layer + linear residual) Trainium2 kernel, 8 NeuronCores.

Strategy (graph/data parallel, per the sharding hint):
  - Nodes are partitioned contiguously across 8 cores (12500 each).
  - Per layer l: each core computes its slice of u_l = g_l @ W_l (bf16),
    AllGathers the slices into a full per-core DRAM table [N,128] bf16,
    then aggregates its own nodes' incoming edges: batched dma_gather of
    u_l[src] rows and a one-hot "selection" matmul that collapses edges
    onto dst nodes, with the GCN norm folded into the selection matrix:
    S[e,p] = norm_e * (r_e == p), built in ONE vector op
    tensor_scalar(iota, r_e, nu_e, is_equal, mult).
  - Aggregation is tiled over static 128-node windows (window w covers
    dst nodes [128w, 128w+128)). dma_gather uses int16 indices, so the
    table is split into <=32768-row regions; per (window, region) there
    are C_q subtiles of 128 edge slots (C_q = global max, so the SPMD
    program is identical on all cores; unused slots are padded with -1
    indices, which the gather skips at zero DMA cost and whose nu=0
    entries zero them out of the selection matmul). All of one window's
    subtile matmuls accumulate in a single PSUM tile; the epilogue
    relu(psum + b) lands feature-major in SBUF staging that feeds the
    next layer's matmul directly.
  - Residual x @ Wfc + bfc overlaps with the collectives; the final
    output is written feature-major [128, 12500] per core and transposed
    on the host.
"""

import os
import sys

import numpy as np

if "/opt/trn_rl_repo" not in sys.path:
    sys.path.insert(0, "/opt/trn_rl_repo")

import ml_dtypes

BF16 = ml_dtypes.bfloat16

P = 128          # partitions / feature dim
D = 128          # feature dim
WW = 256         # aggregation window width (dst nodes per psum tile)
NCORES = 8
REG_ROWS = 32768   # dma_gather int16 index reach per table region
B = 4            # windows per gather batch
WARM_BATCHES = 8   # early batches pad with index 0 (not -1) to init SBUF

_LAST_RESULTS = {}   # test introspection: exec_time etc.


def _regions(N):
    regs = []
    q0 = 0
    while q0 < N:
        regs.append((q0, min(q0 + REG_ROWS, N)))
        q0 += REG_ROWS
    return regs


def _batches(nwin):
    out = []
    w0 = 0
    while w0 < nwin:
        out.append((w0, min(B, nwin - w0)))
        w0 += B
    return out


# --------------------------------------------------------------------------
# Host-side preprocessing
# --------------------------------------------------------------------------

def _core_edges(core, src, dst, nu, npc):
    lo = core * npc
    sel = (dst >= lo) & (dst < lo + npc)
    s_src = src[sel]
    s_dst = dst[sel] - lo
    s_nu = nu[sel]
    order = np.argsort(s_dst, kind="stable")
    return s_src[order], s_dst[order], s_nu[order]


def _count_wr(s_src, s_dst, npc, N):
    """Edge counts per (window, region)."""
    nwin = (npc + WW - 1) // WW
    nreg = len(_regions(N))
    w = s_dst // WW
    q = s_src // REG_ROWS
    cnt = np.zeros((nwin, nreg), np.int64)
    np.add.at(cnt, (w, q), 1)
    return cnt


def _layout(npc, N, C):
    """Static slot layout from per-window caps C [nwin, nreg].

    Returns (nwin, nreg, S, sub0) where sub0[b][q] is the first subtile
    of op (batch b, region q) and subw[w][q] the per-window region base.
    """
    nwin = (npc + WW - 1) // WW
    nreg = C.shape[1]
    batches = _batches(nwin)
    sub0 = []          # per (b, q): first subtile of the op
    wbase = np.zeros((nwin, nreg), np.int64)
    cur = 0
    for b, (w0, bw) in enumerate(batches):
        row = []
        for q in range(nreg):
            row.append(cur)
            for wl in range(bw):
                wbase[w0 + wl, q] = cur
                cur += int(C[w0 + wl, q])
        sub0.append(row)
    return nwin, nreg, int(cur), sub0, wbase


def _finalize_core(s_src, s_dst, s_nu, npc, N, C):
    """Build per-core device arrays for per-window caps C [nwin, nreg].

    Returns idx [P, S*8] int16 (wrapped+replicated, op-major layout),
    r [P, S] f32, nu [P, S] f32.
    """
    regs = _regions(N)
    nwin, nreg, S, sub0, wbase = _layout(npc, N, C)

    slot_r = np.zeros(S * P, dtype=np.float32)
    slot_nu = np.zeros(S * P, dtype=np.float32)
    flat_idx = np.full(S * P, -1, dtype=np.int16)

    w_of = s_dst // WW
    q_of = s_src // REG_ROWS
    batches = _batches(nwin)

    flat_idx[:] = 0
    for w in range(nwin):
        for q in range(nreg):
            m = (w_of == w) & (q_of == q)
            e_src = s_src[m]
            e_dst = s_dst[m]
            e_nu = s_nu[m]
            ne = len(e_src)
            assert ne <= C[w, q] * P
            o = int(wbase[w, q]) * P
            flat_idx[o:o + ne] = (e_src - regs[q][0]).astype(np.int16)
            slot_r[o:o + ne] = (e_dst - w * WW).astype(np.float32)
            slot_nu[o:o + ne] = e_nu
    r = np.ascontiguousarray(slot_r.reshape(S, P).T)
    nnu = np.ascontiguousarray(slot_nu.reshape(S, P).T)

    # wrapped indices: the flat slot order IS the op order (op-major
    # layout); wrap each op's span into 16 partitions, replicate to 128.
    idx16 = np.zeros((16, S * 8), np.int16)
    for b, (w0, bw) in enumerate(batches):
        for q in range(nreg):
            a = sub0[b][q]
            end = a + sum(int(C[w0 + wl, q]) for wl in range(bw))
            ln = (end - a) * P
            if ln == 0:
                continue
            span = flat_idx[a * P: a * P + ln]
            idx16[:, a * 8: a * 8 + ln // 16] = span.reshape(ln // 16, 16).T
    idx = np.ascontiguousarray(np.tile(idx16, (8, 1)))
    return idx, r, nnu


# --------------------------------------------------------------------------
# Device program
# --------------------------------------------------------------------------

def _build_program(N, npc, C):
    from contextlib import ExitStack

    import concourse.bass as bass
    import concourse.tile as tile
    from concourse import bacc, mybir
    from concourse.tile_rust import add_dep_helper

    f32 = mybir.dt.float32
    bf16 = mybir.dt.bfloat16
    i32 = mybir.dt.int32
    i16 = mybir.dt.int16
    AF = mybir.ActivationFunctionType
    ALU = mybir.AluOpType

    nchunks = (npc + P - 1) // P
    regs = _regions(N)
    nwin, nreg, S, sub0, wbase = _layout(npc, N, C)
    batches = _batches(nwin)
    n_ops = len(batches) * nreg
    npc_pad = nchunks * P
    RW = 512
    nrchunks = (npc + RW - 1) // RW

    nc = bacc.Bacc(
        "TRN2",
        target_bir_lowering=False,
        debug=False,
        num_devices=NCORES,
    )

    # ---- I/O ----
    xT_e = nc.dram_tensor("xT", [P, npc], bf16, kind="ExternalInput")
    W1_e = nc.dram_tensor("W1", [D, D], bf16, kind="ExternalInput")
    W2_e = nc.dram_tensor("W2", [D, D], bf16, kind="ExternalInput")
    Wfc_e = nc.dram_tensor("Wfc", [D, D], bf16, kind="ExternalInput")
    b1_e = nc.dram_tensor("b1", [P, 1], f32, kind="ExternalInput")
    b2_e = nc.dram_tensor("b2", [P, 1], f32, kind="ExternalInput")
    bfc_e = nc.dram_tensor("bfc", [P, 1], f32, kind="ExternalInput")
    iota_e = nc.dram_tensor("iota", [P, WW], f32, kind="ExternalInput")
    idx_e = nc.dram_tensor("idx", [P, S * 8], i16, kind="ExternalInput")
    r_e = nc.dram_tensor("r", [P, S], f32, kind="ExternalInput")
    nu_e = nc.dram_tensor("nu", [P, S], f32, kind="ExternalInput")
    out_e = nc.dram_tensor("out", [P, npc], f32, kind="ExternalOutput")

    # ---- internal DRAM ----
    t1 = nc.dram_tensor("table1", [N, D], bf16, addr_space="Shared")
    t2 = nc.dram_tensor("table2", [N, D], bf16, addr_space="Shared")
    bnc1 = nc.dram_tensor("bounce1", [npc_pad, D], bf16)
    bnc2 = nc.dram_tensor("bounce2", [npc_pad, D], bf16)

    rgroups = [list(range(NCORES))]

    with tile.TileContext(nc) as tc, ExitStack() as ctx:
        cpool = ctx.enter_context(tc.tile_pool(name="const", bufs=1))
        gpool = ctx.enter_context(tc.tile_pool(name="gather", bufs=2))
        spool = ctx.enter_context(tc.tile_pool(name="sel", bufs=8))
        stpool = ctx.enter_context(tc.tile_pool(name="stage", bufs=1))
        ckpool = ctx.enter_context(tc.tile_pool(name="chunk", bufs=4))
        pspool = ctx.enter_context(tc.tile_pool(name="ps", bufs=2, space="PSUM"))
        apspool = ctx.enter_context(tc.tile_pool(name="aps", bufs=3, space="PSUM"))
        rpspool = ctx.enter_context(tc.tile_pool(name="rps", bufs=2, space="PSUM"))

        def load_const(ext, shape, dtype):
            t = cpool.tile(shape, dtype, tag=ext.name + "_sb")
            nc.sync.dma_start(out=t[:], in_=ext[:, :])
            return t

        xT = load_const(xT_e, [P, npc], bf16)
        W1 = load_const(W1_e, [D, D], bf16)
        W2 = load_const(W2_e, [D, D], bf16)
        Wfc = load_const(Wfc_e, [D, D], bf16)
        b1 = load_const(b1_e, [P, 1], f32)
        b2 = load_const(b2_e, [P, 1], f32)
        bfc = load_const(bfc_e, [P, 1], f32)
        iot = load_const(iota_e, [P, WW], f32)
        idxm = load_const(idx_e, [P, S * 8], i16)
        rm = load_const(r_e, [P, S], f32)
        num = load_const(nu_e, [P, S], f32)

        stag1 = stpool.tile([P, npc], bf16, tag="stag1")
        stag2 = stpool.tile([P, npc], f32, tag="stag2")

        def production(g_sbuf, W_sb, bounce):
            """u = (g @ W) per 128-node chunk -> bf16 -> bounce DRAM."""
            for c in range(nchunks):
                c0 = c * P
                cn = min(P, npc - c0)
                ps = pspool.tile([P, P], f32, space="PSUM", tag="pps")
                nc.tensor.matmul(
                    out=ps[:cn, :],
                    lhsT=g_sbuf[:, c0:c0 + cn],
                    rhs=W_sb[:],
                    start=True,
                    stop=True,
                )
                ck = ckpool.tile([P, P], bf16, tag="prodck")
                nc.scalar.activation(ck[:cn, :], ps[:cn, :], AF.Copy)
                nc.sync.dma_start(out=bounce[c0:c0 + cn, :], in_=ck[:cn, :])

        def aggregate(table, bias_sb, stag):
            """Gather + selection-matmul + relu epilogue into stag."""
            gmax = [
                max(
                    sum(int(C[w0 + wl, q]) for wl in range(bw))
                    for (w0, bw) in batches
                )
                for q in range(nreg)
            ]
            for b, (w0, bw) in enumerate(batches):
                gtiles = []
                for q in range(nreg):
                    a = sub0[b][q]
                    nsub = sum(int(C[w0 + wl, q]) for wl in range(bw))
                    ln = nsub * P
                    if ln == 0:
                        gtiles.append(None)
                        continue
                    gb = gpool.tile([P, gmax[q] * D], bf16, tag=f"gbuf{q}")
                    nc.gpsimd.dma_gather(
                        gb[:, :nsub * D].rearrange("p (c d) -> p c d", d=D),
                        table[regs[q][0]:regs[q][1], :],
                        idxm[:, a * 8: a * 8 + ln // 16],
                        ln,
                        ln,
                        D,
                        single_packet=False,
                    )
                    gtiles.append((gb, a))
                for wl in range(bw):
                    w = w0 + wl
                    n0 = w * WW
                    wn = min(WW, npc - n0)
                    subw_w = sum(int(C[w, q]) for q in range(nreg))
                    ps = apspool.tile([P, WW], f32, space="PSUM", tag="aps")
                    k = 0
                    for q in range(nreg):
                        if gtiles[q] is None or C[w, q] == 0:
                            continue
                        gb, a = gtiles[q]
                        for s in range(int(C[w, q])):
                            sub = int(wbase[w, q]) + s
                            lsub = sub - a
                            Sp = spool.tile([P, WW], bf16, tag="selm")
                            nc.vector.tensor_scalar(
                                Sp[:],
                                iot[:],
                                rm[:, sub:sub + 1],
                                num[:, sub:sub + 1],
                                ALU.is_equal,
                                ALU.mult,
                            )
                            nc.tensor.matmul(
                                out=ps[:],
                                lhsT=gb[:, lsub * D:(lsub + 1) * D],
                                rhs=Sp[:],
                                start=(k == 0),
                                stop=(k == subw_w - 1),
                            )
                            k += 1
                    nc.scalar.activation(
                        stag[:, n0:n0 + wn],
                        ps[:, :wn],
                        AF.Relu,
                        bias=bias_sb[:, 0:1],
                    )

        # ---------------- layer 1 ----------------
        production(xT, W1, bnc1)
        nc.gpsimd.collective_compute(
            "AllGather",
            ALU.bypass,
            replica_groups=rgroups,
            ins=[bnc1[0:npc, :].opt()],
            outs=[t1[0:NCORES * npc, :].opt()],
        )
        aggregate(t1, b1, stag1)

        # ---------------- layer 2 ----------------
        production(stag1, W2, bnc2)
        nc.gpsimd.collective_compute(
            "AllGather",
            ALU.bypass,
            replica_groups=rgroups,
            ins=[bnc2[0:npc, :].opt()],
            outs=[t2[0:NCORES * npc, :].opt()],
        )
        aggregate(t2, b2, stag2)

        # ---------------- residual + combine ----------------
        for rc in range(nrchunks):
            r0 = rc * RW
            cw = min(RW, npc - r0)
            ps = rpspool.tile([P, RW], f32, space="PSUM", tag="rps")
            nc.tensor.matmul(
                out=ps[:, :cw],
                lhsT=Wfc[:],
                rhs=xT[:, r0:r0 + cw],
                start=True,
                stop=True,
            )
            rb = ckpool.tile([P, RW], f32, tag="resck")
            nc.scalar.activation(
                rb[:, :cw], ps[:, :cw], AF.Identity, bias=bfc[:, 0:1]
            )
            ob = ckpool.tile([P, RW], f32, tag="outck")
            nc.vector.tensor_tensor(
                out=ob[:, :cw],
                in0=rb[:, :cw],
                in1=stag2[:, r0:r0 + cw],
                op=ALU.add,
            )
            nc.sync.dma_start(out=out_e[:, r0:r0 + cw], in_=ob[:, :cw])

    nc.compile()
    return nc


# --------------------------------------------------------------------------
# Entry point
# --------------------------------------------------------------------------

def _prep(x, edge_index, W1, b1, W2, b2, Wfc, bfc):
    N = x.shape[0]
    assert N % NCORES == 0
    npc = N // NCORES

    loop = np.arange(N, dtype=np.int64)
    src = np.concatenate([edge_index[0].astype(np.int64), loop])
    dst = np.concatenate([edge_index[1].astype(np.int64), loop])
    deg = np.bincount(dst, minlength=N).astype(np.float32)
    sigma = np.where(deg > 0, 1.0 / np.sqrt(deg), 0.0).astype(np.float32)
    nu = sigma[src] * sigma[dst]

    cores = [_core_edges(c, src, dst, nu, npc) for c in range(NCORES)]

    nreg = len(_regions(N))
    nwin = (npc + WW - 1) // WW
    maxc = np.zeros((nwin, nreg), np.int64)
    for s_src, s_dst, s_nu in cores:
        cnt = _count_wr(s_src, s_dst, npc, N)
        maxc = np.maximum(maxc, cnt)
    C = (maxc + P - 1) // P

    iota = np.tile(np.arange(WW, dtype=np.float32), (P, 1))
    W1b = np.asarray(W1, np.float32).astype(BF16)
    W2b = np.asarray(W2, np.float32).astype(BF16)
    Wfcb = np.asarray(Wfc, np.float32).astype(BF16)
    b1c = np.asarray(b1, np.float32).reshape(P, 1)
    b2c = np.asarray(b2, np.float32).reshape(P, 1)
    bfcc = np.asarray(bfc, np.float32).reshape(P, 1)

    in_maps = []
    for c in range(NCORES):
        s_src, s_dst, s_nu = cores[c]
        idx, r, nnu = _finalize_core(s_src, s_dst, s_nu, npc, N, C)
        xTc = np.ascontiguousarray(x[c * npc:(c + 1) * npc].T.astype(BF16))
        in_maps.append({
            "xT": xTc,
            "W1": W1b, "W2": W2b, "Wfc": Wfcb,
            "b1": b1c, "b2": b2c, "bfc": bfcc,
            "iota": iota,
            "idx": idx, "r": r, "nu": nnu,
        })
    return in_maps, N, npc, C


def _ensure_ntff_hook():
    """The agent image's antenv lacks axon_hooks; shim it so trace=True
    works (falls back to hookless if the profiling lib is unavailable)."""
    try:
        import antenv.axon_hooks  # noqa: F401
        return
    except ImportError:
        pass
    try:
        import types

        import antenv

        mod = types.ModuleType("antenv.axon_hooks")
        _hook = [None]
        mod.set_axon_ntff_profile_hook = lambda h: _hook.__setitem__(0, h)
        mod.get_axon_ntff_profile_hook = lambda: _hook[0]
        sys.modules["antenv.axon_hooks"] = mod
        antenv.axon_hooks = mod
        try:
            from trn_agent_boot.trn_boot import _ntff_profile_via_ctypes

            mod.set_axon_ntff_profile_hook(
                _ntff_profile_via_ctypes("/opt/axon/libaxon_pjrt.so")
            )
        except Exception:
            pass
    except Exception:
        pass


def kernel(x, edge_index, W1, b1, W2, b2, Wfc, bfc):
    from concourse.bass_utils import run_bass_kernel_spmd

    x = np.asarray(x, np.float32)
    edge_index = np.asarray(edge_index)
    in_maps, N, npc, C = _prep(x, edge_index, W1, b1, W2, b2, Wfc, bfc)
    nc = _build_program(N, npc, C)

    trace = os.environ.get("GNN_TRACE", "0") == "1"
    if trace:
        _ensure_ntff_hook()
    res = run_bass_kernel_spmd(
        nc, in_maps, core_ids=list(range(NCORES)), trace=trace
    )
    _LAST_RESULTS["exec_time_ns"] = res.exec_time_ns
    _LAST_RESULTS["mean_exec_time_ns"] = res.mean_exec_time_ns
    _LAST_RESULTS["trace"] = res.instructions_and_trace

    out = np.concatenate(
        [res.results[c]["out"].T for c in range(NCORES)], axis=0
    )
    return np.ascontiguousarray(out.astype(np.float32))



# revision 20
# speedup vs baseline: 2.5047x; 2.5047x over previous
"""GCN (2-layer + linear residual) Trainium2 kernel, 8 NeuronCores.

Strategy (graph/data parallel, per the sharding hint):
  - Nodes are partitioned contiguously across 8 cores (12500 each).
  - Per layer l: each core computes its slice of u_l = g_l @ W_l (bf16),
    AllGathers the slices into a full per-core DRAM table [N,128] bf16,
    then aggregates its own nodes' incoming edges: batched dma_gather of
    u_l[src] rows and a one-hot "selection" matmul that collapses edges
    onto dst nodes, with the GCN norm folded into the selection matrix.
  - The src table is split into 4 EQUAL 25000-row regions (int16 index
    reach) so per-(window, region) edge counts are balanced and the
    128-slot subtile quantization wastes little.
  - Selection matrices for all subtiles of a (batch, region) op are
    built in 2 wide DVE ops (is_equal then mult) using stride-0
    broadcast access patterns, instead of one tensor_scalar per subtile.
  - dma_gather descriptor generation is spread over SWDGE queues 1-3
    (queue 0 executes synchronously on the GpSimd engine; 1-3 do not).
  - Aggregation windows are WW dst nodes; all of a window's subtile
    matmuls accumulate in one PSUM tile; epilogue relu(psum + b) lands
    feature-major in SBUF staging that feeds the next layer's matmul.
  - Residual x @ Wfc + bfc overlaps; final output is written
    feature-major [128, 12500] per core and transposed on the host.
"""

import os
import sys

import numpy as np

if "/opt/trn_rl_repo" not in sys.path:
    sys.path.insert(0, "/opt/trn_rl_repo")

import ml_dtypes

BF16 = ml_dtypes.bfloat16

P = 128          # partitions / feature dim
D = 128          # feature dim
WW = 256         # aggregation window width (dst nodes per psum tile)
NCORES = 8
REG_ROWS = 25000   # dma_gather int16 region size (4 equal regions)
B = 4            # windows per gather batch
# SWDGE queue schedule: q0 runs inline on the GpSimd engine; q1-3 are
# handled by 2 background workers. Cycle 0,1,2 over ops for a 3-way split,
# emitting async-queue ops before the engine-inline one within each batch.
GQ_CYCLE = (1, 2, 0)

_LAST_RESULTS = {}   # test introspection: exec_time etc.


def _regions(N):
    regs = []
    q0 = 0
    while q0 < N:
        regs.append((q0, min(q0 + REG_ROWS, N)))
        q0 += REG_ROWS
    return regs


def _batches(nwin):
    out = []
    w0 = 0
    while w0 < nwin:
        out.append((w0, min(B, nwin - w0)))
        w0 += B
    return out


# --------------------------------------------------------------------------
# Host-side preprocessing
# --------------------------------------------------------------------------

def _core_edges(core, src, dst, nu, npc):
    lo = core * npc
    sel = (dst >= lo) & (dst < lo + npc)
    s_src = src[sel]
    s_dst = dst[sel] - lo
    s_nu = nu[sel]
    order = np.argsort(s_dst, kind="stable")
    return s_src[order], s_dst[order], s_nu[order]


def _count_wr(s_src, s_dst, npc, N):
    """Edge counts per (window, region)."""
    nwin = (npc + WW - 1) // WW
    nreg = len(_regions(N))
    w = s_dst // WW
    q = s_src // REG_ROWS
    cnt = np.zeros((nwin, nreg), np.int64)
    np.add.at(cnt, (w, q), 1)
    return cnt


def _layout(npc, N, C):
    """Static slot layout from per-window caps C [nwin, nreg].

    Returns (nwin, nreg, S, sub0, wbase) where sub0[b][q] is the first
    subtile of op (batch b, region q) and wbase[w][q] the per-window base.
    """
    nwin = (npc + WW - 1) // WW
    nreg = C.shape[1]
    batches = _batches(nwin)
    sub0 = []          # per (b, q): first subtile of the op
    wbase = np.zeros((nwin, nreg), np.int64)
    cur = 0
    for b, (w0, bw) in enumerate(batches):
        row = []
        for q in range(nreg):
            row.append(cur)
            for wl in range(bw):
                wbase[w0 + wl, q] = cur
                cur += int(C[w0 + wl, q])
        sub0.append(row)
    return nwin, nreg, int(cur), sub0, wbase


def _finalize_core(s_src, s_dst, s_nu, npc, N, C):
    """Build per-core device arrays for per-window caps C [nwin, nreg].

    Returns idx [P, S*8] int16 (wrapped+replicated, op-major layout),
    r [P, S] bf16, nu [P, S] bf16.
    """
    regs = _regions(N)
    nwin, nreg, S, sub0, wbase = _layout(npc, N, C)

    slot_r = np.zeros(S * P, dtype=np.float32)
    slot_nu = np.zeros(S * P, dtype=np.float32)
    flat_idx = np.zeros(S * P, dtype=np.int16)

    w_of = s_dst // WW
    q_of = s_src // REG_ROWS
    batches = _batches(nwin)

    for w in range(nwin):
        for q in range(nreg):
            m = (w_of == w) & (q_of == q)
            e_src = s_src[m]
            e_dst = s_dst[m]
            e_nu = s_nu[m]
            ne = len(e_src)
            assert ne <= C[w, q] * P
            o = int(wbase[w, q]) * P
            flat_idx[o:o + ne] = (e_src - regs[q][0]).astype(np.int16)
            slot_r[o:o + ne] = (e_dst - w * WW).astype(np.float32)
            slot_nu[o:o + ne] = e_nu
    r = np.ascontiguousarray(slot_r.reshape(S, P).T).astype(BF16)
    nnu = np.ascontiguousarray(slot_nu.reshape(S, P).T).astype(BF16)

    # wrapped indices: the flat slot order IS the op order (op-major
    # layout); wrap each op's span into 16 partitions, replicate to 128.
    idx16 = np.zeros((16, S * 8), np.int16)
    for b, (w0, bw) in enumerate(batches):
        for q in range(nreg):
            a = sub0[b][q]
            end = a + sum(int(C[w0 + wl, q]) for wl in range(bw))
            ln = (end - a) * P
            if ln == 0:
                continue
            span = flat_idx[a * P: a * P + ln]
            idx16[:, a * 8: a * 8 + ln // 16] = span.reshape(ln // 16, 16).T
    idx = np.ascontiguousarray(np.tile(idx16, (8, 1)))
    return idx, r, nnu


# --------------------------------------------------------------------------
# Device program
# --------------------------------------------------------------------------

def _build_program(N, npc, C):
    from contextlib import ExitStack

    import concourse.bass as bass
    import concourse.tile as tile
    from concourse import bacc, mybir
    from concourse.bass import broadcast_tensor_aps

    f32 = mybir.dt.float32
    bf16 = mybir.dt.bfloat16
    i16 = mybir.dt.int16
    AF = mybir.ActivationFunctionType
    ALU = mybir.AluOpType

    nchunks = (npc + P - 1) // P
    regs = _regions(N)
    nwin, nreg, S, sub0, wbase = _layout(npc, N, C)
    batches = _batches(nwin)
    npc_pad = nchunks * P
    RW = 512
    nrchunks = (npc + RW - 1) // RW
    # max subtiles of any (batch, region) op: sizes gather + sel buffers
    SMAX = max(
        sum(int(C[w0 + wl, q]) for wl in range(bw))
        for (w0, bw) in batches
        for q in range(nreg)
    )

    nc = bacc.Bacc(
        "TRN2",
        target_bir_lowering=False,
        debug=False,
        num_devices=NCORES,
        num_swdge_queues=4,
    )

    # ---- I/O ----
    xT_e = nc.dram_tensor("xT", [P, npc], bf16, kind="ExternalInput")
    W1_e = nc.dram_tensor("W1", [D, D], bf16, kind="ExternalInput")
    W2_e = nc.dram_tensor("W2", [D, D], bf16, kind="ExternalInput")
    Wfc_e = nc.dram_tensor("Wfc", [D, D], bf16, kind="ExternalInput")
    b1_e = nc.dram_tensor("b1", [P, 1], f32, kind="ExternalInput")
    b2_e = nc.dram_tensor("b2", [P, 1], f32, kind="ExternalInput")
    bfc_e = nc.dram_tensor("bfc", [P, 1], f32, kind="ExternalInput")
    iota_e = nc.dram_tensor("iota", [P, WW], bf16, kind="ExternalInput")
    ident_e = nc.dram_tensor("ident", [P, WW + P], bf16, kind="ExternalInput")
    sg2_e = nc.dram_tensor("sg2", [P, nchunks], f32, kind="ExternalInput")
    idx_e = nc.dram_tensor("idx", [P, S * 8], i16, kind="ExternalInput")
    r_e = nc.dram_tensor("r", [P, S], bf16, kind="ExternalInput")
    nu_e = nc.dram_tensor("nu", [P, S], bf16, kind="ExternalInput")
    out_e = nc.dram_tensor("out", [P, npc], f32, kind="ExternalOutput")

    # ---- internal DRAM ----
    t1 = nc.dram_tensor("table1", [N, D], bf16, addr_space="Shared")
    t2 = nc.dram_tensor("table2", [N, D], bf16, addr_space="Shared")
    bnc1 = nc.dram_tensor("bounce1", [npc_pad, D], bf16)
    bnc2 = nc.dram_tensor("bounce2", [npc_pad, D], bf16)

    rgroups = [list(range(NCORES))]

    with tile.TileContext(nc) as tc, ExitStack() as ctx:
        cpool = ctx.enter_context(tc.tile_pool(name="const", bufs=1))
        gpool = ctx.enter_context(tc.tile_pool(name="gather", bufs=2))
        spool = ctx.enter_context(tc.tile_pool(name="sel", bufs=4))
        stpool = ctx.enter_context(tc.tile_pool(name="stage", bufs=1))
        ckpool = ctx.enter_context(tc.tile_pool(name="chunk", bufs=2))
        pdpool = ctx.enter_context(tc.tile_pool(name="prodst", bufs=2))
        pspool = ctx.enter_context(tc.tile_pool(name="ps", bufs=2, space="PSUM"))
        apspool = ctx.enter_context(tc.tile_pool(name="aps", bufs=3, space="PSUM"))
        rpspool = ctx.enter_context(tc.tile_pool(name="rps", bufs=2, space="PSUM"))

        def load_const(ext, shape, dtype):
            t = cpool.tile(shape, dtype, tag=ext.name + "_sb")
            nc.sync.dma_start(out=t[:], in_=ext[:, :])
            return t

        xT = load_const(xT_e, [P, npc], bf16)
        W1 = load_const(W1_e, [D, D], bf16)
        W2 = load_const(W2_e, [D, D], bf16)
        Wfc = load_const(Wfc_e, [D, D], bf16)
        b1 = load_const(b1_e, [P, 1], f32)
        b2 = load_const(b2_e, [P, 1], f32)
        bfc = load_const(bfc_e, [P, 1], f32)
        iot = load_const(iota_e, [P, WW], bf16)
        ident = load_const(ident_e, [P, WW + P], bf16)
        sg2 = load_const(sg2_e, [P, nchunks], f32)
        idxm = load_const(idx_e, [P, S * 8], i16)
        rm = load_const(r_e, [P, S], bf16)
        num = load_const(nu_e, [P, S], bf16)

        stag1 = stpool.tile([P, npc], bf16, tag="stag1")
        stag2 = stpool.tile([P, npc], bf16, tag="stag2")
        # self-loop terms sigma_i^2 * u_l[i], node-major per chunk
        sstag = stpool.tile([P, nchunks * D], bf16, tag="sstag")

        PB = 4   # production chunks per bounce write

        def production(g_sbuf, W_sb, bounce):
            """u = (g @ W) per 128-node chunk -> bf16 -> bounce DRAM."""
            for g0 in range(0, nchunks, PB):
                gn = min(PB, nchunks - g0)
                st = pdpool.tile([P, PB * P], bf16, tag="prodst")
                for j in range(gn):
                    c = g0 + j
                    c0 = c * P
                    cn = min(P, npc - c0)
                    ps = pspool.tile([P, P], f32, space="PSUM", tag="pps")
                    nc.tensor.matmul(
                        out=ps[:cn, :],
                        lhsT=g_sbuf[:, c0:c0 + cn],
                        rhs=W_sb[:],
                        start=True,
                        stop=True,
                    )
                    nc.scalar.activation(
                        st[:cn, j * P:(j + 1) * P], ps[:cn, :], AF.Copy
                    )
                    nc.scalar.activation(
                        sstag[:cn, c * D:(c + 1) * D], ps[:cn, :], AF.Copy,
                        scale=sg2[:cn, c:c + 1],
                    )
                nc.sync.dma_start(
                    out=bounce[g0 * P: g0 * P + gn * P, :].rearrange(
                        "(j p) d -> p j d", p=P
                    ),
                    in_=st[:, : gn * P].rearrange("p (j d) -> p j d", d=D),
                )

        def aggregate(table, bias_sb, stag):
            """Gather + selection-matmul + relu epilogue into stag."""
            for b, (w0, bw) in enumerate(batches):
                gtiles = [None] * nreg
                stiles = [None] * nreg
                qnums = [GQ_CYCLE[(b * nreg + q) % 3] for q in range(nreg)]
                order = sorted(range(nreg), key=lambda q: qnums[q] == 0)
                for q in order:
                    a = sub0[b][q]
                    nsub = sum(int(C[w0 + wl, q]) for wl in range(bw))
                    ln = nsub * P
                    if ln == 0:
                        continue
                    gb = gpool.tile([P, SMAX * D], bf16, tag=f"gbuf{q}")
                    nc.gpsimd.dma_gather(
                        gb[:, :nsub * D].rearrange("p (c d) -> p c d", d=D),
                        table[regs[q][0]:regs[q][1], :],
                        idxm[:, a * 8: a * 8 + ln // 16],
                        ln,
                        ln,
                        D,
                        single_packet=False,
                        queue_num=qnums[q],
                    )
                    gtiles[q] = (gb, a)
                    # batched selection build: S[p, s, j] =
                    #   nu[p, a+s] * (iota[j] == r[p, a+s])
                    Sb = spool.tile([P, SMAX * WW], bf16, tag="selm")
                    s3 = Sb[:, :nsub * WW].rearrange("p (s w) -> p s w", w=WW)
                    i3 = iot[:].rearrange("p (s w) -> p s w", s=1)
                    r3 = rm[:, a:a + nsub].rearrange("p (s w) -> p s w", w=1)
                    n3 = num[:, a:a + nsub].rearrange("p (s w) -> p s w", w=1)
                    i3b, r3b = broadcast_tensor_aps(i3, r3)
                    s3b, n3b = broadcast_tensor_aps(s3, n3)
                    nc.vector.tensor_tensor(
                        out=s3, in0=i3b, in1=r3b, op=ALU.is_equal
                    )
                    nc.vector.tensor_tensor(
                        out=s3, in0=s3b, in1=n3b, op=ALU.mult
                    )
                    stiles[q] = (Sb, a)
                for wl in range(bw):
                    w = w0 + wl
                    n0 = w * WW
                    wn = min(WW, npc - n0)
                    chunks_w = [
                        c for c in range(n0 // P, min((n0 + WW) // P, nchunks))
                    ]
                    subw_w = sum(int(C[w, q]) for q in range(nreg))
                    nmm = subw_w + len(chunks_w)
                    ps = apspool.tile([P, WW], f32, space="PSUM", tag="aps")
                    k = 0
                    # self-loop terms: sstag chunk^T scattered onto the
                    # diagonal via identity matmul
                    # self-loop terms via shifted-identity matmuls over the
                    # full window width (uniform start/stop accumulation)
                    for c in chunks_w:
                        c0 = c * P
                        cn = min(P, npc - c0)
                        o = c0 - n0
                        nc.tensor.matmul(
                            out=ps[:],
                            lhsT=sstag[:cn, c * D:(c + 1) * D],
                            rhs=ident[:cn, P - o:P - o + WW],
                            start=(k == 0),
                            stop=(k == nmm - 1),
                        )
                        k += 1
                    for q in range(nreg):
                        if gtiles[q] is None or C[w, q] == 0:
                            continue
                        gb, a = gtiles[q]
                        Sb, _ = stiles[q]
                        for s in range(int(C[w, q])):
                            sub = int(wbase[w, q]) + s
                            lsub = sub - a
                            nc.tensor.matmul(
                                out=ps[:],
                                lhsT=gb[:, lsub * D:(lsub + 1) * D],
                                rhs=Sb[:, lsub * WW:(lsub + 1) * WW],
                                start=False,
                                stop=(k == nmm - 1),
                            )
                            k += 1
                    nc.scalar.activation(
                        stag[:, n0:n0 + wn],
                        ps[:, :wn],
                        AF.Relu,
                        bias=bias_sb[:, 0:1],
                    )

        # ---------------- layer 1 ----------------
        production(xT, W1, bnc1)
        nc.gpsimd.collective_compute(
            "AllGather",
            ALU.bypass,
            replica_groups=rgroups,
            ins=[bnc1[0:npc, :].opt()],
            outs=[t1[0:NCORES * npc, :].opt()],
        )
        aggregate(t1, b1, stag1)

        # ---------------- layer 2 ----------------
        production(stag1, W2, bnc2)
        nc.gpsimd.collective_compute(
            "AllGather",
            ALU.bypass,
            replica_groups=rgroups,
            ins=[bnc2[0:npc, :].opt()],
            outs=[t2[0:NCORES * npc, :].opt()],
        )
        aggregate(t2, b2, stag2)

        # ---------------- residual + combine ----------------
        for rc in range(nrchunks):
            r0 = rc * RW
            cw = min(RW, npc - r0)
            ps = rpspool.tile([P, RW], f32, space="PSUM", tag="rps")
            nc.tensor.matmul(
                out=ps[:, :cw],
                lhsT=Wfc[:],
                rhs=xT[:, r0:r0 + cw],
                start=True,
                stop=True,
            )
            rb = ckpool.tile([P, RW], f32, tag="resck")
            nc.scalar.activation(
                rb[:, :cw], ps[:, :cw], AF.Identity, bias=bfc[:, 0:1]
            )
            ob = ckpool.tile([P, RW], f32, tag="outck")
            nc.vector.tensor_tensor(
                out=ob[:, :cw],
                in0=rb[:, :cw],
                in1=stag2[:, r0:r0 + cw],
                op=ALU.add,
            )
            nc.sync.dma_start(out=out_e[:, r0:r0 + cw], in_=ob[:, :cw])

    nc.compile()
    return nc


# --------------------------------------------------------------------------
# Entry point
# --------------------------------------------------------------------------

def _prep(x, edge_index, W1, b1, W2, b2, Wfc, bfc):
    N = x.shape[0]
    assert N % NCORES == 0
    npc = N // NCORES

    loop = np.arange(N, dtype=np.int64)
    src = edge_index[0].astype(np.int64)
    dst = edge_index[1].astype(np.int64)
    # deg includes the appended self-loops (matching the reference)...
    deg = (np.bincount(dst, minlength=N) + 1).astype(np.float32)
    sigma = np.where(deg > 0, 1.0 / np.sqrt(deg), 0.0).astype(np.float32)
    nu = sigma[src] * sigma[dst]
    # ...but the self-loop edges themselves are NOT gathered: their
    # contribution sigma_i^2 * u[i] is core-local and is added on-device
    # from the production output via a scaled identity matmul.
    sig2 = (sigma * sigma).astype(np.float32)

    cores = [_core_edges(c, src, dst, nu, npc) for c in range(NCORES)]

    nreg = len(_regions(N))
    nwin = (npc + WW - 1) // WW
    maxc = np.zeros((nwin, nreg), np.int64)
    for s_src, s_dst, s_nu in cores:
        cnt = _count_wr(s_src, s_dst, npc, N)
        maxc = np.maximum(maxc, cnt)
    C = (maxc + P - 1) // P

    iota = np.tile(np.arange(WW, dtype=np.float32), (P, 1)).astype(BF16)
    # shifted identity: Q[p, P + p] = 1; slice [P-o : P-o+WW] places the
    # diagonal at column offset o of a WW-wide window
    ident = np.zeros((P, WW + P), np.float32)
    ident[np.arange(P), np.arange(P) + P] = 1.0
    ident = ident.astype(BF16)
    nchunks = (npc + P - 1) // P
    W1b = np.asarray(W1, np.float32).astype(BF16)
    W2b = np.asarray(W2, np.float32).astype(BF16)
    Wfcb = np.asarray(Wfc, np.float32).astype(BF16)
    b1c = np.asarray(b1, np.float32).reshape(P, 1)
    b2c = np.asarray(b2, np.float32).reshape(P, 1)
    bfcc = np.asarray(bfc, np.float32).reshape(P, 1)

    in_maps = []
    for c in range(NCORES):
        s_src, s_dst, s_nu = cores[c]
        idx, r, nnu = _finalize_core(s_src, s_dst, s_nu, npc, N, C)
        xTc = np.ascontiguousarray(x[c * npc:(c + 1) * npc].T.astype(BF16))
        sg2pad = np.zeros(nchunks * P, np.float32)
        sg2pad[:npc] = sig2[c * npc:(c + 1) * npc]
        sg2c = np.ascontiguousarray(sg2pad.reshape(nchunks, P).T)
        in_maps.append({
            "xT": xTc,
            "W1": W1b, "W2": W2b, "Wfc": Wfcb,
            "b1": b1c, "b2": b2c, "bfc": bfcc,
            "iota": iota, "ident": ident, "sg2": sg2c,
            "idx": idx, "r": r, "nu": nnu,
        })
    return in_maps, N, npc, C


def _ensure_ntff_hook():
    """The agent image's antenv lacks axon_hooks; shim it so trace=True
    works (falls back to hookless if the profiling lib is unavailable)."""
    try:
        import antenv.axon_hooks  # noqa: F401
        return
    except ImportError:
        pass
    try:
        import types

        import antenv

        mod = types.ModuleType("antenv.axon_hooks")
        _hook = [None]
        mod.set_axon_ntff_profile_hook = lambda h: _hook.__setitem__(0, h)
        mod.get_axon_ntff_profile_hook = lambda: _hook[0]
        sys.modules["antenv.axon_hooks"] = mod
        antenv.axon_hooks = mod
        try:
            from trn_agent_boot.trn_boot import _ntff_profile_via_ctypes

            mod.set_axon_ntff_profile_hook(
                _ntff_profile_via_ctypes("/opt/axon/libaxon_pjrt.so")
            )
        except Exception:
            pass
    except Exception:
        pass


def kernel(x, edge_index, W1, b1, W2, b2, Wfc, bfc):
    from concourse.bass_utils import run_bass_kernel_spmd

    x = np.asarray(x, np.float32)
    edge_index = np.asarray(edge_index)
    in_maps, N, npc, C = _prep(x, edge_index, W1, b1, W2, b2, Wfc, bfc)
    nc = _build_program(N, npc, C)

    trace = os.environ.get("GNN_TRACE", "0") == "1"
    if trace:
        _ensure_ntff_hook()
    res = run_bass_kernel_spmd(
        nc, in_maps, core_ids=list(range(NCORES)), trace=trace
    )
    _LAST_RESULTS["exec_time_ns"] = res.exec_time_ns
    _LAST_RESULTS["mean_exec_time_ns"] = res.mean_exec_time_ns
    _LAST_RESULTS["trace"] = res.instructions_and_trace

    out = np.concatenate(
        [res.results[c]["out"].T for c in range(NCORES)], axis=0
    )
    return np.ascontiguousarray(out.astype(np.float32))


# revision 23
# speedup vs baseline: 3.2104x; 1.2817x over previous
"""GCN (2-layer + linear residual) Trainium2 kernel, 8 NeuronCores.

Strategy (graph/data parallel, per the sharding hint):
  - Nodes are partitioned contiguously across 8 cores (12500 each).
  - Per layer l: each core computes its slice of u_l = g_l @ W_l (bf16),
    AllGathers the slices into a full per-core DRAM table [N,128] bf16,
    then aggregates its own nodes' incoming edges: batched dma_gather of
    u_l[src] rows and a one-hot "selection" matmul that collapses edges
    onto dst nodes, with the GCN norm folded into the selection matrix.
  - The src table is split into 4 EQUAL 25000-row regions (int16 index
    reach) so per-(window, region) edge counts are balanced and the
    128-slot subtile quantization wastes little.
  - Selection matrices for all subtiles of a (batch, region) op are
    built in 2 wide DVE ops (is_equal then mult) using stride-0
    broadcast access patterns, instead of one tensor_scalar per subtile.
  - dma_gather descriptor generation is spread over SWDGE queues 1-3
    (queue 0 executes synchronously on the GpSimd engine; 1-3 do not).
  - Aggregation windows are WW dst nodes; all of a window's subtile
    matmuls accumulate in one PSUM tile; epilogue relu(psum + b) lands
    feature-major in SBUF staging that feeds the next layer's matmul.
  - Residual x @ Wfc + bfc overlaps; final output is written
    feature-major [128, 12500] per core and transposed on the host.
"""

import os
import sys

import numpy as np

if "/opt/trn_rl_repo" not in sys.path:
    sys.path.insert(0, "/opt/trn_rl_repo")

import ml_dtypes

BF16 = ml_dtypes.bfloat16

P = 128          # partitions / feature dim
D = 128          # feature dim
WW = 128         # aggregation window width (dst nodes per psum tile)
NCORES = 8
REG_ROWS = 25000   # dma_gather int16 region size (4 equal regions)
B = 8            # windows per gather batch
# SWDGE queue per region: q0 runs inline on the GpSimd engine; q1-3 feed 2
# background workers. Distinct queues within a batch (a queue holds one
# in-flight op); async-queue ops are emitted before the engine-inline one.
GQUEUES = (1, 2, 3, 0)

_LAST_RESULTS = {}   # test introspection: exec_time etc.


def _regions(N):
    regs = []
    q0 = 0
    while q0 < N:
        regs.append((q0, min(q0 + REG_ROWS, N)))
        q0 += REG_ROWS
    return regs


def _batches(nwin):
    out = []
    w0 = 0
    while w0 < nwin:
        out.append((w0, min(B, nwin - w0)))
        w0 += B
    return out


# --------------------------------------------------------------------------
# Host-side preprocessing
# --------------------------------------------------------------------------

def _core_edges(core, src, dst, nu, npc):
    lo = core * npc
    sel = (dst >= lo) & (dst < lo + npc)
    s_src = src[sel]
    s_dst = dst[sel] - lo
    s_nu = nu[sel]
    order = np.argsort(s_dst, kind="stable")
    return s_src[order], s_dst[order], s_nu[order]


def _count_wr(s_src, s_dst, npc, N):
    """Edge counts per (window, region)."""
    nwin = (npc + WW - 1) // WW
    nreg = len(_regions(N))
    w = s_dst // WW
    q = s_src // REG_ROWS
    cnt = np.zeros((nwin, nreg), np.int64)
    np.add.at(cnt, (w, q), 1)
    return cnt


def _layout(npc, N, C):
    """Static slot layout from per-window caps C [nwin, nreg].

    Returns (nwin, nreg, S, sub0, wbase) where sub0[b][q] is the first
    subtile of op (batch b, region q) and wbase[w][q] the per-window base.
    """
    nwin = (npc + WW - 1) // WW
    nreg = C.shape[1]
    batches = _batches(nwin)
    sub0 = []          # per (b, q): first subtile of the op
    wbase = np.zeros((nwin, nreg), np.int64)
    cur = 0
    for b, (w0, bw) in enumerate(batches):
        row = []
        for q in range(nreg):
            row.append(cur)
            for wl in range(bw):
                wbase[w0 + wl, q] = cur
                cur += int(C[w0 + wl, q])
        sub0.append(row)
    return nwin, nreg, int(cur), sub0, wbase


def _finalize_core(s_src, s_dst, s_nu, npc, N, C):
    """Build per-core device arrays for per-window caps C [nwin, nreg].

    Returns idx [P, S*8] int16 (wrapped+replicated, op-major layout),
    r [P, S] bf16, nu [P, S] bf16.
    """
    regs = _regions(N)
    nwin, nreg, S, sub0, wbase = _layout(npc, N, C)

    slot_r = np.zeros(S * P, dtype=np.float32)
    slot_nu = np.zeros(S * P, dtype=np.float32)
    flat_idx = np.zeros(S * P, dtype=np.int16)

    w_of = s_dst // WW
    q_of = s_src // REG_ROWS
    batches = _batches(nwin)

    for w in range(nwin):
        for q in range(nreg):
            m = (w_of == w) & (q_of == q)
            e_src = s_src[m]
            e_dst = s_dst[m]
            e_nu = s_nu[m]
            ne = len(e_src)
            assert ne <= C[w, q] * P
            o = int(wbase[w, q]) * P
            flat_idx[o:o + ne] = (e_src - regs[q][0]).astype(np.int16)
            slot_r[o:o + ne] = (e_dst - w * WW).astype(np.float32)
            slot_nu[o:o + ne] = e_nu
    r = np.ascontiguousarray(slot_r.reshape(S, P).T).astype(BF16)
    nnu = np.ascontiguousarray(slot_nu.reshape(S, P).T).astype(BF16)

    # wrapped indices: the flat slot order IS the op order (op-major
    # layout); wrap each op's span into 16 partitions, replicate to 128.
    idx16 = np.zeros((16, S * 8), np.int16)
    for b, (w0, bw) in enumerate(batches):
        for q in range(nreg):
            a = sub0[b][q]
            end = a + sum(int(C[w0 + wl, q]) for wl in range(bw))
            ln = (end - a) * P
            if ln == 0:
                continue
            span = flat_idx[a * P: a * P + ln]
            idx16[:, a * 8: a * 8 + ln // 16] = span.reshape(ln // 16, 16).T
    idx = np.ascontiguousarray(np.tile(idx16, (8, 1)))
    return idx, r, nnu


# --------------------------------------------------------------------------
# Device program
# --------------------------------------------------------------------------

def _build_program(N, npc, C):
    from contextlib import ExitStack

    import concourse.bass as bass
    import concourse.tile as tile
    from concourse import bacc, mybir
    from concourse.bass import broadcast_tensor_aps

    f32 = mybir.dt.float32
    bf16 = mybir.dt.bfloat16
    i16 = mybir.dt.int16
    AF = mybir.ActivationFunctionType
    ALU = mybir.AluOpType

    nchunks = (npc + P - 1) // P
    regs = _regions(N)
    nwin, nreg, S, sub0, wbase = _layout(npc, N, C)
    batches = _batches(nwin)
    npc_pad = nchunks * P
    RW = 512
    nrchunks = (npc + RW - 1) // RW
    # max subtiles of any (batch, region) op: sizes gather + sel buffers
    SMAX = max(
        sum(int(C[w0 + wl, q]) for wl in range(bw))
        for (w0, bw) in batches
        for q in range(nreg)
    )

    nc = bacc.Bacc(
        "TRN2",
        target_bir_lowering=False,
        debug=False,
        num_devices=NCORES,
        num_swdge_queues=4,
    )

    # ---- I/O ----
    xT_e = nc.dram_tensor("xT", [P, npc], bf16, kind="ExternalInput")
    W1_e = nc.dram_tensor("W1", [D, D], bf16, kind="ExternalInput")
    W2_e = nc.dram_tensor("W2", [D, D], bf16, kind="ExternalInput")
    Wfc_e = nc.dram_tensor("Wfc", [D, D], bf16, kind="ExternalInput")
    b1_e = nc.dram_tensor("b1", [P, 1], f32, kind="ExternalInput")
    b2_e = nc.dram_tensor("b2", [P, 1], f32, kind="ExternalInput")
    bfc_e = nc.dram_tensor("bfc", [P, 1], f32, kind="ExternalInput")
    iota_e = nc.dram_tensor("iota", [P, WW], bf16, kind="ExternalInput")
    ident_e = nc.dram_tensor("ident", [P, WW + P], bf16, kind="ExternalInput")
    sg2_e = nc.dram_tensor("sg2", [P, nchunks], f32, kind="ExternalInput")
    idx_e = nc.dram_tensor("idx", [P, S * 8], i16, kind="ExternalInput")
    r_e = nc.dram_tensor("r", [P, S], bf16, kind="ExternalInput")
    nu_e = nc.dram_tensor("nu", [P, S], bf16, kind="ExternalInput")
    out_e = nc.dram_tensor("out", [P, npc], f32, kind="ExternalOutput")

    # ---- internal DRAM ----
    t1 = nc.dram_tensor("table1", [N, D], bf16, addr_space="Shared")
    t2 = nc.dram_tensor("table2", [N, D], bf16, addr_space="Shared")
    bnc1 = nc.dram_tensor("bounce1", [npc_pad, D], bf16)
    bnc2 = nc.dram_tensor("bounce2", [npc_pad, D], bf16)

    rgroups = [list(range(NCORES))]

    with tile.TileContext(nc) as tc, ExitStack() as ctx:
        cpool = ctx.enter_context(tc.tile_pool(name="const", bufs=1))
        gpool = ctx.enter_context(tc.tile_pool(name="gather", bufs=2))
        spool = ctx.enter_context(tc.tile_pool(name="sel", bufs=4))
        stpool = ctx.enter_context(tc.tile_pool(name="stage", bufs=1))
        ckpool = ctx.enter_context(tc.tile_pool(name="chunk", bufs=2))
        pdpool = ctx.enter_context(tc.tile_pool(name="prodst", bufs=2))
        pspool = ctx.enter_context(tc.tile_pool(name="ps", bufs=2, space="PSUM"))
        apspool = ctx.enter_context(tc.tile_pool(name="aps", bufs=4, space="PSUM"))
        rpspool = ctx.enter_context(tc.tile_pool(name="rps", bufs=2, space="PSUM"))

        def load_const(ext, shape, dtype):
            t = cpool.tile(shape, dtype, tag=ext.name + "_sb")
            nc.sync.dma_start(out=t[:], in_=ext[:, :])
            return t

        xT = load_const(xT_e, [P, npc], bf16)
        W1 = load_const(W1_e, [D, D], bf16)
        W2 = load_const(W2_e, [D, D], bf16)
        Wfc = load_const(Wfc_e, [D, D], bf16)
        b1 = load_const(b1_e, [P, 1], f32)
        b2 = load_const(b2_e, [P, 1], f32)
        bfc = load_const(bfc_e, [P, 1], f32)
        iot = load_const(iota_e, [P, WW], bf16)
        ident = load_const(ident_e, [P, WW + P], bf16)
        sg2 = load_const(sg2_e, [P, nchunks], f32)
        idxm = load_const(idx_e, [P, S * 8], i16)
        rm = load_const(r_e, [P, S], bf16)
        num = load_const(nu_e, [P, S], bf16)

        stag1 = stpool.tile([P, npc], bf16, tag="stag1")
        stag2 = stpool.tile([P, npc], bf16, tag="stag2")
        # self-loop terms sigma_i^2 * u_l[i], node-major per chunk
        sstag = stpool.tile([P, nchunks * D], bf16, tag="sstag")

        PB = 4   # production chunks per bounce write

        def production(g_sbuf, W_sb, bounce):
            """u = (g @ W) per 128-node chunk -> bf16 -> bounce DRAM."""
            for g0 in range(0, nchunks, PB):
                gn = min(PB, nchunks - g0)
                st = pdpool.tile([P, PB * P], bf16, tag="prodst")
                for j in range(gn):
                    c = g0 + j
                    c0 = c * P
                    cn = min(P, npc - c0)
                    ps = pspool.tile([P, P], f32, space="PSUM", tag="pps")
                    nc.tensor.matmul(
                        out=ps[:cn, :],
                        lhsT=g_sbuf[:, c0:c0 + cn],
                        rhs=W_sb[:],
                        start=True,
                        stop=True,
                    )
                    nc.scalar.activation(
                        st[:cn, j * P:(j + 1) * P], ps[:cn, :], AF.Copy
                    )
                    nc.scalar.activation(
                        sstag[:cn, c * D:(c + 1) * D], ps[:cn, :], AF.Copy,
                        scale=sg2[:cn, c:c + 1],
                    )
                nc.sync.dma_start(
                    out=bounce[g0 * P: g0 * P + gn * P, :].rearrange(
                        "(j p) d -> p j d", p=P
                    ),
                    in_=st[:, : gn * P].rearrange("p (j d) -> p j d", d=D),
                )

        def aggregate(table, bias_sb, stag):
            """Gather + selection-matmul + relu epilogue into stag."""
            for b, (w0, bw) in enumerate(batches):
                gtiles = [None] * nreg
                stiles = [None] * nreg
                qnums = [GQUEUES[q % 4] for q in range(nreg)]
                order = sorted(range(nreg), key=lambda q: qnums[q] == 0)
                for q in order:
                    a = sub0[b][q]
                    nsub = sum(int(C[w0 + wl, q]) for wl in range(bw))
                    ln = nsub * P
                    if ln == 0:
                        continue
                    gb = gpool.tile([P, SMAX * D], bf16, tag=f"gbuf{q}")
                    nc.gpsimd.dma_gather(
                        gb[:, :nsub * D].rearrange("p (c d) -> p c d", d=D),
                        table[regs[q][0]:regs[q][1], :],
                        idxm[:, a * 8: a * 8 + ln // 16],
                        ln,
                        ln,
                        D,
                        single_packet=False,
                        queue_num=qnums[q],
                    )
                    gtiles[q] = (gb, a)
                    # batched selection build: S[p, s, j] =
                    #   nu[p, a+s] * (iota[j] == r[p, a+s])
                    Sb = spool.tile([P, SMAX * WW], bf16, tag="selm")
                    s3 = Sb[:, :nsub * WW].rearrange("p (s w) -> p s w", w=WW)
                    i3 = iot[:].rearrange("p (s w) -> p s w", s=1)
                    r3 = rm[:, a:a + nsub].rearrange("p (s w) -> p s w", w=1)
                    n3 = num[:, a:a + nsub].rearrange("p (s w) -> p s w", w=1)
                    i3b, r3b = broadcast_tensor_aps(i3, r3)
                    s3b, n3b = broadcast_tensor_aps(s3, n3)
                    nc.vector.tensor_tensor(
                        out=s3, in0=i3b, in1=r3b, op=ALU.is_equal
                    )
                    nc.vector.tensor_tensor(
                        out=s3, in0=s3b, in1=n3b, op=ALU.mult
                    )
                    stiles[q] = (Sb, a)
                for wl in range(bw):
                    w = w0 + wl
                    n0 = w * WW
                    wn = min(WW, npc - n0)
                    chunks_w = [
                        c for c in range(n0 // P, min((n0 + WW) // P, nchunks))
                    ]
                    subw_w = sum(int(C[w, q]) for q in range(nreg))
                    nmm = subw_w + len(chunks_w)
                    ps = apspool.tile([P, WW], f32, space="PSUM", tag="aps")
                    k = 0
                    # self-loop terms: sstag chunk^T scattered onto the
                    # diagonal via identity matmul
                    # self-loop terms via shifted-identity matmuls over the
                    # full window width (uniform start/stop accumulation)
                    for c in chunks_w:
                        c0 = c * P
                        cn = min(P, npc - c0)
                        o = c0 - n0
                        nc.tensor.matmul(
                            out=ps[:],
                            lhsT=sstag[:cn, c * D:(c + 1) * D],
                            rhs=ident[:cn, P - o:P - o + WW],
                            start=(k == 0),
                            stop=(k == nmm - 1),
                        )
                        k += 1
                    for q in range(nreg):
                        if gtiles[q] is None or C[w, q] == 0:
                            continue
                        gb, a = gtiles[q]
                        Sb, _ = stiles[q]
                        for s in range(int(C[w, q])):
                            sub = int(wbase[w, q]) + s
                            lsub = sub - a
                            nc.tensor.matmul(
                                out=ps[:],
                                lhsT=gb[:, lsub * D:(lsub + 1) * D],
                                rhs=Sb[:, lsub * WW:(lsub + 1) * WW],
                                start=False,
                                stop=(k == nmm - 1),
                            )
                            k += 1
                    nc.scalar.activation(
                        stag[:, n0:n0 + wn],
                        ps[:, :wn],
                        AF.Relu,
                        bias=bias_sb[:, 0:1],
                    )

        # ---------------- layer 1 ----------------
        production(xT, W1, bnc1)
        nc.gpsimd.collective_compute(
            "AllGather",
            ALU.bypass,
            replica_groups=rgroups,
            ins=[bnc1[0:npc, :].opt()],
            outs=[t1[0:NCORES * npc, :].opt()],
        )
        aggregate(t1, b1, stag1)

        # ---------------- layer 2 ----------------
        production(stag1, W2, bnc2)
        nc.gpsimd.collective_compute(
            "AllGather",
            ALU.bypass,
            replica_groups=rgroups,
            ins=[bnc2[0:npc, :].opt()],
            outs=[t2[0:NCORES * npc, :].opt()],
        )
        aggregate(t2, b2, stag2)

        # ---------------- residual + combine ----------------
        for rc in range(nrchunks):
            r0 = rc * RW
            cw = min(RW, npc - r0)
            ps = rpspool.tile([P, RW], f32, space="PSUM", tag="rps")
            nc.tensor.matmul(
                out=ps[:, :cw],
                lhsT=Wfc[:],
                rhs=xT[:, r0:r0 + cw],
                start=True,
                stop=True,
            )
            rb = ckpool.tile([P, RW], f32, tag="resck")
            nc.scalar.activation(
                rb[:, :cw], ps[:, :cw], AF.Identity, bias=bfc[:, 0:1]
            )
            ob = ckpool.tile([P, RW], f32, tag="outck")
            nc.vector.tensor_tensor(
                out=ob[:, :cw],
                in0=rb[:, :cw],
                in1=stag2[:, r0:r0 + cw],
                op=ALU.add,
            )
            nc.sync.dma_start(out=out_e[:, r0:r0 + cw], in_=ob[:, :cw])

    nc.compile()
    return nc


# --------------------------------------------------------------------------
# Entry point
# --------------------------------------------------------------------------

def _prep(x, edge_index, W1, b1, W2, b2, Wfc, bfc):
    N = x.shape[0]
    assert N % NCORES == 0
    npc = N // NCORES

    loop = np.arange(N, dtype=np.int64)
    src = edge_index[0].astype(np.int64)
    dst = edge_index[1].astype(np.int64)
    # deg includes the appended self-loops (matching the reference)...
    deg = (np.bincount(dst, minlength=N) + 1).astype(np.float32)
    sigma = np.where(deg > 0, 1.0 / np.sqrt(deg), 0.0).astype(np.float32)
    nu = sigma[src] * sigma[dst]
    # ...but the self-loop edges themselves are NOT gathered: their
    # contribution sigma_i^2 * u[i] is core-local and is added on-device
    # from the production output via a scaled identity matmul.
    sig2 = (sigma * sigma).astype(np.float32)

    cores = [_core_edges(c, src, dst, nu, npc) for c in range(NCORES)]

    nreg = len(_regions(N))
    nwin = (npc + WW - 1) // WW
    maxc = np.zeros((nwin, nreg), np.int64)
    for s_src, s_dst, s_nu in cores:
        cnt = _count_wr(s_src, s_dst, npc, N)
        maxc = np.maximum(maxc, cnt)
    C = (maxc + P - 1) // P

    iota = np.tile(np.arange(WW, dtype=np.float32), (P, 1)).astype(BF16)
    # shifted identity: Q[p, P + p] = 1; slice [P-o : P-o+WW] places the
    # diagonal at column offset o of a WW-wide window
    ident = np.zeros((P, WW + P), np.float32)
    ident[np.arange(P), np.arange(P) + P] = 1.0
    ident = ident.astype(BF16)
    nchunks = (npc + P - 1) // P
    W1b = np.asarray(W1, np.float32).astype(BF16)
    W2b = np.asarray(W2, np.float32).astype(BF16)
    Wfcb = np.asarray(Wfc, np.float32).astype(BF16)
    b1c = np.asarray(b1, np.float32).reshape(P, 1)
    b2c = np.asarray(b2, np.float32).reshape(P, 1)
    bfcc = np.asarray(bfc, np.float32).reshape(P, 1)

    in_maps = []
    for c in range(NCORES):
        s_src, s_dst, s_nu = cores[c]
        idx, r, nnu = _finalize_core(s_src, s_dst, s_nu, npc, N, C)
        xTc = np.ascontiguousarray(x[c * npc:(c + 1) * npc].T.astype(BF16))
        sg2pad = np.zeros(nchunks * P, np.float32)
        sg2pad[:npc] = sig2[c * npc:(c + 1) * npc]
        sg2c = np.ascontiguousarray(sg2pad.reshape(nchunks, P).T)
        in_maps.append({
            "xT": xTc,
            "W1": W1b, "W2": W2b, "Wfc": Wfcb,
            "b1": b1c, "b2": b2c, "bfc": bfcc,
            "iota": iota, "ident": ident, "sg2": sg2c,
            "idx": idx, "r": r, "nu": nnu,
        })
    return in_maps, N, npc, C


def _ensure_ntff_hook():
    """The agent image's antenv lacks axon_hooks; shim it so trace=True
    works (falls back to hookless if the profiling lib is unavailable)."""
    try:
        import antenv.axon_hooks  # noqa: F401
        return
    except ImportError:
        pass
    try:
        import types

        import antenv

        mod = types.ModuleType("antenv.axon_hooks")
        _hook = [None]
        mod.set_axon_ntff_profile_hook = lambda h: _hook.__setitem__(0, h)
        mod.get_axon_ntff_profile_hook = lambda: _hook[0]
        sys.modules["antenv.axon_hooks"] = mod
        antenv.axon_hooks = mod
        try:
            from trn_agent_boot.trn_boot import _ntff_profile_via_ctypes

            mod.set_axon_ntff_profile_hook(
                _ntff_profile_via_ctypes("/opt/axon/libaxon_pjrt.so")
            )
        except Exception:
            pass
    except Exception:
        pass


def kernel(x, edge_index, W1, b1, W2, b2, Wfc, bfc):
    from concourse.bass_utils import run_bass_kernel_spmd

    x = np.asarray(x, np.float32)
    edge_index = np.asarray(edge_index)
    in_maps, N, npc, C = _prep(x, edge_index, W1, b1, W2, b2, Wfc, bfc)
    nc = _build_program(N, npc, C)

    trace = os.environ.get("GNN_TRACE", "0") == "1"
    if trace:
        _ensure_ntff_hook()
    res = run_bass_kernel_spmd(
        nc, in_maps, core_ids=list(range(NCORES)), trace=trace
    )
    _LAST_RESULTS["exec_time_ns"] = res.exec_time_ns
    _LAST_RESULTS["mean_exec_time_ns"] = res.mean_exec_time_ns
    _LAST_RESULTS["trace"] = res.instructions_and_trace

    out = np.concatenate(
        [res.results[c]["out"].T for c in range(NCORES)], axis=0
    )
    return np.ascontiguousarray(out.astype(np.float32))
